# revision 28
# baseline (speedup 1.0000x reference)
"""Trainium2 Bass kernel for the RNN-T style Joint network:

    out[b,t,u,v] = sum_k tanh(enc_p[b,t,k] + dec_p[b,u,k] + b1[k]) * W2[v,k] + b2[v]
    enc_p = h_enc @ W1[:, :H].T ; dec_p = h_dec @ W1[:, H:].T

Sharding: data-parallel over B across 8 NeuronCores (B == 8, one batch row per
core). Weights are replicated. No collectives needed.

Per-core pipeline (one NeuronCore):
  warmup (PE): dummy N=512 matmuls during the input-DMA wait keep the HAM
      clock gate at K=8/8 from the start.
  input DMA: W1 packed host-side by GEMM1 k-tile (kk); chunks balanced over
      the sync/scalar/gpsimd queues (each queue's transfers serialize at
      ~85GB/s and only start ~8us in, so the critical path is per-queue
      bytes, not issue count).
  GEMM1 (PE): per kk as its weights land: enc_pT [HID, T] and dec_pT [HID, U]
      on 2 rotating PSUM banks, b1 folded via ScalarE bias during bf16 evac.
      Build adds for chunks 0/1 interleave per-kk so GEMM2 starts early.
  broadcast-add (VectorE, bf16): pre[j, t'*64+u] = encbT[j, t] + decT[j, u],
      one tensor_add per [128, 1024] chunk via stride-0 broadcast APs.
  tanh (ScalarE): bf16 -> bf16 hT tiles (stationary operand of GEMM2).
  GEMM2 (PE, bf16): 1280 N=512 matmuls (~216ns each, the roofline term) with
      5-K-tile accumulation into fp32 PSUM (bf16 PSUM is TRN3-only).
  b2 + evac (VectorE): PSUM + b2 -> bf16 out tile [128, 2048].
  DMA out: one 512KB store per two mt blocks, round-robin over three queues;
      the final pair splits into two parallel single-block stores.

Output is written bf16 and upcast to fp32 on the host (adds ~0.1% rms noise;
well inside the 2e-2 gate) to halve the HBM write traffic and SBUF footprint.
"""

import numpy as np
import ml_dtypes

B, T, U, H = 8, 256, 64, 512
HID, V = 640, 1024
TU = T * U  # 16384
N_CORES = 8
N_CHUNKS = TU // 1024  # 16 chunks of 16 t-values x 64 u-values
KK = HID // 128  # 5 K-tiles

BF16 = ml_dtypes.bfloat16

_CACHE = {}


def _build_bass():
    import concourse.bass as bass
    import concourse.tile as tile
    from concourse import bacc, mybir

    f32 = mybir.dt.float32
    bf16 = mybir.dt.bfloat16
    Tanh = mybir.ActivationFunctionType.Tanh

    nc = bacc.Bacc("TRN2", target_bir_lowering=False, debug=False,
                   num_devices=N_CORES)

    # W1 enc/dec halves packed as [128, KK, 4, 128] -> [128, KK*512]
    w1eP = nc.dram_tensor("w1eP", [128, KK * 512], bf16, kind="ExternalInput").ap()
    w1dP = nc.dram_tensor("w1dP", [128, KK * 512], bf16, kind="ExternalInput").ap()
    hencP = nc.dram_tensor("hencP", [128, 4 * T], bf16, kind="ExternalInput").ap()
    hdecP = nc.dram_tensor("hdecP", [128, 4 * U], bf16, kind="ExternalInput").ap()
    w2P = nc.dram_tensor("w2P", [128, KK * V], bf16, kind="ExternalInput").ap()
    b1P = nc.dram_tensor("b1P", [128, KK], f32, kind="ExternalInput").ap()
    b2P = nc.dram_tensor("b2P", [1, V], bf16, kind="ExternalInput").ap()
    out = nc.dram_tensor("out", [TU, V], bf16, kind="ExternalOutput").ap()

    def bcast3(ap2d, mid):
        """[P, N] AP -> [P, mid, N] with a stride-0 middle dim."""
        return bass.AP(tensor=ap2d.tensor, offset=ap2d.offset,
                       ap=[ap2d.ap[0], [0, mid], ap2d.ap[1]])

    def repeat3(ap2d, inner):
        """[P, N] AP -> [P, N, inner] with a stride-0 inner dim."""
        return bass.AP(tensor=ap2d.tensor, offset=ap2d.offset,
                       ap=[ap2d.ap[0], ap2d.ap[1], [0, inner]])

    with tile.TileContext(nc) as tc:
        with (
            tc.tile_pool(name="consts", bufs=1) as consts,
            tc.tile_pool(name="psum", bufs=1, space="PSUM") as psum,
            tc.tile_pool(name="prep", bufs=2) as prep,
            tc.tile_pool(name="hTp", bufs=4) as hTp,
            tc.tile_pool(name="outp", bufs=6) as outp,
        ):
            # ---- scratch + PE warmup during the input-DMA wait ----
            wk = consts.tile([128, 512], bf16, tag="wk", name="wk")
            nc.vector.memset(wk, 0)

            def warm_mms(n, label):
                for i in range(n):
                    pw = psum.tile([128, 512], f32, tag="g1", bufs=2,
                                   name=f"warm_{label}_{i}")
                    nc.tensor.matmul(pw, lhsT=wk[:, :128], rhs=wk,
                                     start=True, stop=True)

            warm_mms(6, "pre")

            # ---- input DMAs: balanced over the three DMA-capable queues ----
            henc_all = consts.tile([128, 4 * T], bf16, tag="henc", name="henc")
            hdec_all = consts.tile([128, 4 * U], bf16, tag="hdec", name="hdec")
            b1_all = consts.tile([128, KK], f32, tag="b1", name="b1")
            w1e_all = consts.tile([128, KK * 512], bf16, tag="w1e", name="w1e")
            w1d_all = consts.tile([128, KK * 512], bf16, tag="w1d", name="w1d")
            w2_all = consts.tile([128, KK * V], bf16, tag="w2", name="w2")
            b2_t = consts.tile([128, V], bf16, tag="b2", name="b2")

            def w1e_dma(q, kk):
                q.dma_start(out=w1e_all[:, kk * 512:(kk + 1) * 512],
                            in_=w1eP[:, kk * 512:(kk + 1) * 512])

            def w1d_dma(q, kk):
                q.dma_start(out=w1d_all[:, kk * 512:(kk + 1) * 512],
                            in_=w1dP[:, kk * 512:(kk + 1) * 512])

            def w2_dma(q, kk):
                q.dma_start(out=w2_all[:, kk * V:(kk + 1) * V],
                            in_=w2P[:, kk * V:(kk + 1) * V])

            b2_row = consts.tile([1, V], bf16, tag="b2row", name="b2row")

            # sync queue (~1.02MB): GEMM1 inputs first, then w2 kk0/kk3
            nc.sync.dma_start(out=henc_all[:, 2 * T:], in_=hencP[:, 2 * T:])
            w1e_dma(nc.sync, 0)
            w1e_dma(nc.sync, 2)
            w1d_dma(nc.sync, 2)
            w2_dma(nc.sync, 0)
            w2_dma(nc.sync, 3)
            # scalar queue (~1.02MB)
            nc.scalar.dma_start(out=henc_all[:, :2 * T], in_=hencP[:, :2 * T])
            w1e_dma(nc.scalar, 1)
            w1e_dma(nc.scalar, 3)
            w1d_dma(nc.scalar, 3)
            w2_dma(nc.scalar, 1)
            w2_dma(nc.scalar, 4)
            # gpsimd queue (~0.84MB); b2 ships as one row, replicated on-chip
            nc.gpsimd.dma_start(out=b2_row, in_=b2P[:, :])
            nc.gpsimd.dma_start(out=b1_all, in_=b1P[:, :])
            nc.gpsimd.dma_start(out=hdec_all, in_=hdecP[:, :])
            w1d_dma(nc.gpsimd, 0)
            w1d_dma(nc.gpsimd, 1)
            w1e_dma(nc.gpsimd, 4)
            w1d_dma(nc.gpsimd, 4)
            w2_dma(nc.gpsimd, 2)
            # replicate b2 across partitions: ones[1,128].T @ b2row[1,V]
            ones_t = consts.tile([1, 128], bf16, tag="ones", name="ones")
            nc.vector.memset(ones_t, 1.0)
            for vc in range(2):
                pb = psum.tile([128, 512], f32, tag="g1", bufs=2,
                               name=f"b2rep{vc}")
                nc.tensor.matmul(pb, lhsT=ones_t,
                                 rhs=b2_row[:, vc * 512:(vc + 1) * 512],
                                 start=True, stop=True)
                nc.scalar.copy(out=b2_t[:, vc * 512:(vc + 1) * 512], in_=pb)

            # ---- GEMM1 per kk (as weights land) + early builds ----
            encbT = []
            decT = []
            hts0 = []
            hts1 = []

            def build_one(c, kk, hts):
                pre = prep.tile([128, 1024], bf16, tag=f"pre{kk}",
                                name=f"pre{c}_{kk}", bufs=2)
                pre_ap = pre[:, :]
                out3 = bass.AP(tensor=pre_ap.tensor, offset=pre_ap.offset,
                               ap=[pre_ap.ap[0], [64, 16], [1, 64]])
                nc.vector.tensor_add(
                    out=out3,
                    in0=bcast3(decT[kk][:, :], 16),
                    in1=repeat3(encbT[kk][:, c * 16:(c + 1) * 16], 64),
                )
                ht = hTp.tile([128, 1024], bf16, tag=f"hT{kk}",
                              name=f"hT{c}_{kk}", bufs=4)
                nc.scalar.activation(out=ht, in_=pre, func=Tanh)
                hts.append(ht)

            for kk in range(KK):
                pe_ = psum.tile([128, 512], f32, tag="g1", bufs=2,
                                name=f"g1e{kk}")
                for k in range(4):
                    nc.tensor.matmul(
                        pe_[:, :T],
                        lhsT=w1e_all[:, kk * 512 + k * 128:
                                     kk * 512 + (k + 1) * 128],
                        rhs=henc_all[:, k * T:(k + 1) * T],
                        start=(k == 0), stop=(k == 3),
                    )
                e_ = consts.tile([128, T], bf16, tag=f"encbT{kk}",
                                 name=f"encbT{kk}")
                nc.scalar.add(out=e_, in_=pe_[:, :T], add=b1_all[:, kk:kk + 1])
                encbT.append(e_)
                pd_ = psum.tile([128, 512], f32, tag="g1", bufs=2,
                                name=f"g1d{kk}")
                for k in range(4):
                    nc.tensor.matmul(
                        pd_[:, :U],
                        lhsT=w1d_all[:, kk * 512 + k * 128:
                                     kk * 512 + (k + 1) * 128],
                        rhs=hdec_all[:, k * U:(k + 1) * U],
                        start=(k == 0), stop=(k == 3),
                    )
                d_ = consts.tile([128, U], bf16, tag=f"decT{kk}",
                                 name=f"decT{kk}")
                nc.scalar.copy(out=d_, in_=pd_[:, :U])
                decT.append(d_)
                # keep the PE warm while the next kk's weights arrive
                warm_mms(1, f"g1_{kk}")

            # chunk-major: chunk 0's adds run first so GEMM2 opens ASAP
            for kk in range(KK):
                build_one(0, kk, hts0)
            for kk in range(KK):
                build_one(1, kk, hts1)
            hT_by_chunk = {0: hts0, 1: hts1}

            def emit_build(c):
                hts = []
                for kk in range(KK):
                    build_one(c, kk, hts)
                hT_by_chunk[c] = hts

            # contiguous bridge burst right before GEMM2: >=3us of
            # back-to-back matmuls so GEMM2 opens at K=8/8
            warm_mms(8, "bridge")

            for c in range(N_CHUNKS):
                hts = hT_by_chunk.pop(c)
                last_c = c == N_CHUNKS - 1
                for pair in range(4):
                    if pair == 2 and c + 2 < N_CHUNKS:
                        emit_build(c + 2)
                    last_pair = last_c and pair == 3
                    ot = outp.tile([128, 2 * V], bf16, tag="out",
                                   name=f"out{c}_{pair}")
                    if last_pair:
                        # Four per-vc [128,512] PSUM tiles (the g1 tag is
                        # free after GEMM1) so each evac depends only on its
                        # own 5-matmul group, not the whole tile. ScalarE
                        # pre-writes b2 into the vc=1 tiles (their groups
                        # accumulate onto it via has_written, still set from
                        # the banks' previous use) and evacuates them in
                        # parallel with DVE after the final matmul.
                        for half in range(2):
                            mt = pair * 2 + half
                            for vc in range(2):
                                pv = psum.tile([128, 512], f32, tag="g1",
                                               bufs=2,
                                               name=f"psL_{mt}_{vc}")
                                if vc == 1:
                                    nc.scalar.copy(out=pv, in_=b2_t[:, 512:])
                                for kk in range(KK):
                                    nc.tensor.matmul(
                                        pv,
                                        lhsT=hts[kk][:, mt * 128:(mt + 1) * 128],
                                        rhs=w2_all[:, kk * V + vc * 512:
                                                   kk * V + (vc + 1) * 512],
                                        start=(kk == 0 and vc == 0),
                                        stop=(kk == KK - 1),
                                    )
                                if vc == 0:
                                    nc.vector.tensor_add(
                                        out=ot[:, half * V:half * V + 512],
                                        in0=pv, in1=b2_t[:, :512])
                                else:
                                    nc.scalar.copy(
                                        out=ot[:, half * V + 512:
                                               (half + 1) * V],
                                        in_=pv)
                    else:
                        for half in range(2):
                            mt = pair * 2 + half
                            ps2 = psum.tile([128, 1024], f32, tag="g2",
                                            bufs=3,
                                            name=f"ps2_{c}_{mt}")
                            for kk in range(KK):
                                for vc in range(2):
                                    nc.tensor.matmul(
                                        ps2[:, vc * 512:(vc + 1) * 512],
                                        lhsT=hts[kk][:, mt * 128:(mt + 1) * 128],
                                        rhs=w2_all[:, kk * V + vc * 512:
                                                   kk * V + (vc + 1) * 512],
                                        start=(kk == 0), stop=(kk == KK - 1),
                                    )
                            nc.vector.tensor_add(
                                out=ot[:, half * V:(half + 1) * V],
                                in0=ps2, in1=b2_t)
                    r0 = c * 1024 + pair * 256
                    ot_ap = ot[:, :]
                    if last_pair:
                        # four small stores on alternating queues so the
                        # final transfer isn't one long serial DMA
                        qs = (nc.sync, nc.gpsimd, nc.scalar, nc.sync)
                        for j in range(4):
                            half, vc = divmod(j, 2)
                            i2 = bass.AP(tensor=ot_ap.tensor,
                                         offset=ot_ap.offset + half * V + vc * 512,
                                         ap=[ot_ap.ap[0], [1, 512]])
                            o2 = out[r0 + half * 128:r0 + (half + 1) * 128,
                                     vc * 512:(vc + 1) * 512]
                            qs[j].dma_start(out=o2, in_=i2)
                    else:
                        in3 = bass.AP(tensor=ot_ap.tensor, offset=ot_ap.offset,
                                      ap=[ot_ap.ap[0], [V, 2], [1, V]])
                        o3 = bass.AP(tensor=out.tensor, offset=r0 * V,
                                     ap=[[V, 128], [128 * V, 2], [1, V]])
                        q = (nc.sync, nc.gpsimd, nc.scalar)[(c * 4 + pair) % 3]
                        q.dma_start(out=o3, in_=in3)

    nc.finalize()
    return nc


def _get_nc():
    if "nc" not in _CACHE:
        _CACHE["nc"] = _build_bass()
    return _CACHE["nc"]


def _pack_w1_half(w1_half):
    """[HID, H] -> [128, KK*4*128] where chunk kk holds the 4 k-tiles of
    lhsT (partitions = the 2H contraction dim)."""
    arr = w1_half.reshape(KK, 128, 4, 128)  # [kk, q(out), k, p(contract)]
    return np.ascontiguousarray(
        arr.transpose(3, 0, 2, 1).reshape(128, KK * 4 * 128))


def _make_in_maps(h_enc, h_dec, W1, b1, W2, b2):
    h_enc = np.asarray(h_enc, dtype=np.float32)
    h_dec = np.asarray(h_dec, dtype=np.float32)
    W1 = np.asarray(W1, dtype=np.float32)
    b1 = np.asarray(b1, dtype=np.float32)
    W2 = np.asarray(W2, dtype=np.float32)
    b2 = np.asarray(b2, dtype=np.float32)

    w1eP = _pack_w1_half(W1[:, :H]).astype(BF16)
    w1dP = _pack_w1_half(W1[:, H:]).astype(BF16)
    w2T = np.ascontiguousarray(W2.T)
    w2P = np.concatenate([w2T[kk * 128:(kk + 1) * 128, :] for kk in range(KK)],
                         axis=1).astype(BF16)
    b1P = np.ascontiguousarray(b1.reshape(KK, 128).T)  # [128, KK] f32
    b2P = np.ascontiguousarray(b2.reshape(1, V)).astype(BF16)

    in_maps = []
    for b in range(N_CORES):
        hencT = np.ascontiguousarray(h_enc[b].T)  # [H, T]
        hencP = hencT.reshape(4, 128, T).transpose(1, 0, 2).reshape(
            128, 4 * T).astype(BF16)
        hdecT = np.ascontiguousarray(h_dec[b].T)  # [H, U]
        hdecP = hdecT.reshape(4, 128, U).transpose(1, 0, 2).reshape(
            128, 4 * U).astype(BF16)
        in_maps.append({
            "hencP": np.ascontiguousarray(hencP),
            "hdecP": np.ascontiguousarray(hdecP),
            "w1eP": w1eP,
            "w1dP": w1dP,
            "w2P": w2P,
            "b1P": b1P,
            "b2P": b2P,
        })
    return in_maps


def _run(in_maps, **kwargs):
    from concourse import bass_utils
    nc = _get_nc()
    return bass_utils.run_bass_kernel_spmd(
        nc, in_maps, core_ids=list(range(N_CORES)), **kwargs)


def kernel(h_enc, h_dec, W1, b1, W2, b2):
    in_maps = _make_in_maps(h_enc, h_dec, W1, b1, W2, b2)
    res = _run(in_maps)
    outs = [r["out"].reshape(T, U, V).astype(np.float32)
            for r in res.results]
    return np.stack(outs, axis=0)


# revision 29
# speedup vs baseline: 1.0012x; 1.0012x over previous
"""Trainium2 Bass kernel for the RNN-T style Joint network:

    out[b,t,u,v] = sum_k tanh(enc_p[b,t,k] + dec_p[b,u,k] + b1[k]) * W2[v,k] + b2[v]
    enc_p = h_enc @ W1[:, :H].T ; dec_p = h_dec @ W1[:, H:].T

Sharding: data-parallel over B across 8 NeuronCores (B == 8, one batch row per
core). Weights are replicated. No collectives needed.

Per-core pipeline (one NeuronCore):
  warmup (PE): dummy N=512 matmuls during the input-DMA wait keep the HAM
      clock gate at K=8/8 from the start.
  input DMA: W1 packed host-side by GEMM1 k-tile (kk); chunks balanced over
      the sync/scalar/gpsimd queues (each queue's transfers serialize at
      ~85GB/s and only start ~8us in, so the critical path is per-queue
      bytes, not issue count).
  GEMM1 (PE): per kk as its weights land: enc_pT [HID, T] and dec_pT [HID, U]
      on 2 rotating PSUM banks, b1 folded via ScalarE bias during bf16 evac.
      Build adds for chunks 0/1 interleave per-kk so GEMM2 starts early.
  broadcast-add (VectorE, bf16): pre[j, t'*64+u] = encbT[j, t] + decT[j, u],
      one tensor_add per [128, 1024] chunk via stride-0 broadcast APs.
  tanh (ScalarE): bf16 -> bf16 hT tiles (stationary operand of GEMM2).
  GEMM2 (PE, bf16): 1280 N=512 matmuls (~216ns each, the roofline term) with
      5-K-tile accumulation into fp32 PSUM (bf16 PSUM is TRN3-only).
  b2 + evac (VectorE): PSUM + b2 -> bf16 out tile [128, 2048].
  DMA out: one 512KB store per two mt blocks, round-robin over three queues;
      the final pair uses per-vc PSUM tiles, DVE/ScalarE parallel evacuation,
      and four small parallel stores to minimize the post-matmul drain.

Output is written bf16 and upcast to fp32 on the host (adds ~0.1% rms noise;
well inside the 2e-2 gate) to halve the HBM write traffic and SBUF footprint.
"""

import numpy as np
import ml_dtypes

B, T, U, H = 8, 256, 64, 512
HID, V = 640, 1024
TU = T * U  # 16384
N_CORES = 8
N_CHUNKS = TU // 1024  # 16 chunks of 16 t-values x 64 u-values
KK = HID // 128  # 5 K-tiles

BF16 = ml_dtypes.bfloat16

_CACHE = {}


def _build_bass():
    import concourse.bass as bass
    import concourse.tile as tile
    from concourse import bacc, mybir

    f32 = mybir.dt.float32
    bf16 = mybir.dt.bfloat16
    Tanh = mybir.ActivationFunctionType.Tanh

    nc = bacc.Bacc("TRN2", target_bir_lowering=False, debug=False,
                   num_devices=N_CORES)

    # W1 enc/dec halves packed as [128, KK, 4, 128] -> [128, KK*512]
    w1eP = nc.dram_tensor("w1eP", [128, KK * 512], bf16, kind="ExternalInput").ap()
    w1dP = nc.dram_tensor("w1dP", [128, KK * 512], bf16, kind="ExternalInput").ap()
    hencP = nc.dram_tensor("hencP", [128, 4 * T], bf16, kind="ExternalInput").ap()
    hdecP = nc.dram_tensor("hdecP", [128, 4 * U], bf16, kind="ExternalInput").ap()
    w2P = nc.dram_tensor("w2P", [128, KK * V], bf16, kind="ExternalInput").ap()
    b1P = nc.dram_tensor("b1P", [128, KK], f32, kind="ExternalInput").ap()
    b2P = nc.dram_tensor("b2P", [1, V], bf16, kind="ExternalInput").ap()
    out = nc.dram_tensor("out", [TU, V], bf16, kind="ExternalOutput").ap()

    def bcast3(ap2d, mid):
        """[P, N] AP -> [P, mid, N] with a stride-0 middle dim."""
        return bass.AP(tensor=ap2d.tensor, offset=ap2d.offset,
                       ap=[ap2d.ap[0], [0, mid], ap2d.ap[1]])

    def repeat3(ap2d, inner):
        """[P, N] AP -> [P, N, inner] with a stride-0 inner dim."""
        return bass.AP(tensor=ap2d.tensor, offset=ap2d.offset,
                       ap=[ap2d.ap[0], ap2d.ap[1], [0, inner]])

    with tile.TileContext(nc) as tc:
        with (
            tc.tile_pool(name="consts", bufs=1) as consts,
            tc.tile_pool(name="psum", bufs=1, space="PSUM") as psum,
            tc.tile_pool(name="prep", bufs=2) as prep,
            tc.tile_pool(name="hTp", bufs=4) as hTp,
            tc.tile_pool(name="outp", bufs=6) as outp,
        ):
            # ---- scratch + PE warmup during the input-DMA wait ----
            wk = consts.tile([128, 512], bf16, tag="wk", name="wk")
            nc.vector.memset(wk, 0)

            def warm_mms(n, label):
                for i in range(n):
                    pw = psum.tile([128, 512], f32, tag="g1", bufs=2,
                                   name=f"warm_{label}_{i}")
                    nc.tensor.matmul(pw, lhsT=wk[:, :128], rhs=wk,
                                     start=True, stop=True)

            warm_mms(6, "pre")

            # ---- input DMAs: balanced over the three DMA-capable queues ----
            henc_all = consts.tile([128, 4 * T], bf16, tag="henc", name="henc")
            hdec_all = consts.tile([128, 4 * U], bf16, tag="hdec", name="hdec")
            b1_all = consts.tile([128, KK], f32, tag="b1", name="b1")
            w1e_all = consts.tile([128, KK * 512], bf16, tag="w1e", name="w1e")
            w1d_all = consts.tile([128, KK * 512], bf16, tag="w1d", name="w1d")
            w2_all = consts.tile([128, KK * V], bf16, tag="w2", name="w2")
            b2_t = consts.tile([128, V], bf16, tag="b2", name="b2")

            def w1e_dma(q, kk):
                q.dma_start(out=w1e_all[:, kk * 512:(kk + 1) * 512],
                            in_=w1eP[:, kk * 512:(kk + 1) * 512])

            def w1d_dma(q, kk):
                q.dma_start(out=w1d_all[:, kk * 512:(kk + 1) * 512],
                            in_=w1dP[:, kk * 512:(kk + 1) * 512])

            def w2_dma(q, kk):
                q.dma_start(out=w2_all[:, kk * V:(kk + 1) * V],
                            in_=w2P[:, kk * V:(kk + 1) * V])

            b2_row = consts.tile([1, V], bf16, tag="b2row", name="b2row")

            # sync queue (~1.02MB): GEMM1 inputs first, then w2 kk0/kk3
            nc.sync.dma_start(out=henc_all[:, 2 * T:], in_=hencP[:, 2 * T:])
            w1e_dma(nc.sync, 0)
            w1e_dma(nc.sync, 2)
            w1d_dma(nc.sync, 2)
            w2_dma(nc.sync, 0)
            w2_dma(nc.sync, 3)
            # scalar queue (~1.02MB)
            nc.scalar.dma_start(out=henc_all[:, :2 * T], in_=hencP[:, :2 * T])
            w1e_dma(nc.scalar, 1)
            w1e_dma(nc.scalar, 3)
            w1d_dma(nc.scalar, 3)
            w2_dma(nc.scalar, 1)
            w2_dma(nc.scalar, 4)
            # gpsimd queue (~0.84MB); b2 ships as one row, replicated on-chip
            nc.gpsimd.dma_start(out=b2_row, in_=b2P[:, :])
            nc.gpsimd.dma_start(out=b1_all, in_=b1P[:, :])
            nc.gpsimd.dma_start(out=hdec_all, in_=hdecP[:, :])
            w1d_dma(nc.gpsimd, 0)
            w1d_dma(nc.gpsimd, 1)
            w1e_dma(nc.gpsimd, 4)
            w1d_dma(nc.gpsimd, 4)
            w2_dma(nc.gpsimd, 2)
            # replicate b2 across partitions: ones[1,128].T @ b2row[1,V]
            ones_t = consts.tile([1, 128], bf16, tag="ones", name="ones")
            nc.vector.memset(ones_t, 1.0)
            for vc in range(2):
                pb = psum.tile([128, 512], f32, tag="g1", bufs=2,
                               name=f"b2rep{vc}")
                nc.tensor.matmul(pb, lhsT=ones_t,
                                 rhs=b2_row[:, vc * 512:(vc + 1) * 512],
                                 start=True, stop=True)
                nc.scalar.copy(out=b2_t[:, vc * 512:(vc + 1) * 512], in_=pb)

            # ---- GEMM1 per kk (as weights land) + early builds ----
            encbT = []
            decT = []
            hts0 = []
            hts1 = []

            def build_one(c, kk, hts):
                pre = prep.tile([128, 1024], bf16, tag=f"pre{kk}",
                                name=f"pre{c}_{kk}", bufs=2)
                pre_ap = pre[:, :]
                out3 = bass.AP(tensor=pre_ap.tensor, offset=pre_ap.offset,
                               ap=[pre_ap.ap[0], [64, 16], [1, 64]])
                nc.vector.tensor_add(
                    out=out3,
                    in0=bcast3(decT[kk][:, :], 16),
                    in1=repeat3(encbT[kk][:, c * 16:(c + 1) * 16], 64),
                )
                ht = hTp.tile([128, 1024], bf16, tag=f"hT{kk}",
                              name=f"hT{c}_{kk}", bufs=4)
                nc.scalar.activation(out=ht, in_=pre, func=Tanh)
                hts.append(ht)

            for kk in range(KK):
                pe_ = psum.tile([128, 512], f32, tag="g1", bufs=2,
                                name=f"g1e{kk}")
                for k in range(4):
                    nc.tensor.matmul(
                        pe_[:, :T],
                        lhsT=w1e_all[:, kk * 512 + k * 128:
                                     kk * 512 + (k + 1) * 128],
                        rhs=henc_all[:, k * T:(k + 1) * T],
                        start=(k == 0), stop=(k == 3),
                    )
                e_ = consts.tile([128, T], bf16, tag=f"encbT{kk}",
                                 name=f"encbT{kk}")
                nc.scalar.add(out=e_, in_=pe_[:, :T], add=b1_all[:, kk:kk + 1])
                encbT.append(e_)
                pd_ = psum.tile([128, 512], f32, tag="g1", bufs=2,
                                name=f"g1d{kk}")
                for k in range(4):
                    nc.tensor.matmul(
                        pd_[:, :U],
                        lhsT=w1d_all[:, kk * 512 + k * 128:
                                     kk * 512 + (k + 1) * 128],
                        rhs=hdec_all[:, k * U:(k + 1) * U],
                        start=(k == 0), stop=(k == 3),
                    )
                d_ = consts.tile([128, U], bf16, tag=f"decT{kk}",
                                 name=f"decT{kk}")
                nc.scalar.copy(out=d_, in_=pd_[:, :U])
                decT.append(d_)
                # keep the PE warm while the next kk's weights arrive
                warm_mms(1, f"g1_{kk}")

            # chunk-major: chunk 0's adds run first so GEMM2 opens ASAP
            for kk in range(KK):
                build_one(0, kk, hts0)
            for kk in range(KK):
                build_one(1, kk, hts1)
            hT_by_chunk = {0: hts0, 1: hts1}

            def emit_build(c):
                hts = []
                for kk in range(KK):
                    build_one(c, kk, hts)
                hT_by_chunk[c] = hts

            # contiguous bridge burst right before GEMM2: >=3us of
            # back-to-back matmuls so GEMM2 opens at K=8/8
            warm_mms(8, "bridge")

            for c in range(N_CHUNKS):
                hts = hT_by_chunk.pop(c)
                last_c = c == N_CHUNKS - 1
                for pair in range(4):
                    if pair == 2 and c + 2 < N_CHUNKS:
                        emit_build(c + 2)
                    last_pair = last_c and pair == 3
                    ot = outp.tile([128, 2 * V], bf16, tag="out",
                                   name=f"out{c}_{pair}")
                    if last_pair:
                        # Four per-vc [128,512] PSUM tiles (the g1 tag is
                        # free after GEMM1) so each evac depends only on its
                        # own 5-matmul group, not the whole tile. ScalarE
                        # pre-writes b2 into the vc=1 tiles (their groups
                        # accumulate onto it via has_written, still set from
                        # the banks' previous use) and evacuates them in
                        # parallel with DVE after the final matmul.
                        for half in range(2):
                            mt = pair * 2 + half
                            for vc in range(2):
                                pv = psum.tile([128, 512], f32, tag="g1",
                                               bufs=2,
                                               name=f"psL_{mt}_{vc}")
                                if vc == 1:
                                    nc.scalar.copy(out=pv, in_=b2_t[:, 512:])
                                for kk in range(KK):
                                    nc.tensor.matmul(
                                        pv,
                                        lhsT=hts[kk][:, mt * 128:(mt + 1) * 128],
                                        rhs=w2_all[:, kk * V + vc * 512:
                                                   kk * V + (vc + 1) * 512],
                                        start=(kk == 0 and vc == 0),
                                        stop=(kk == KK - 1),
                                    )
                                if vc == 0:
                                    nc.vector.tensor_add(
                                        out=ot[:, half * V:half * V + 512],
                                        in0=pv, in1=b2_t[:, :512])
                                else:
                                    nc.scalar.copy(
                                        out=ot[:, half * V + 512:
                                               (half + 1) * V],
                                        in_=pv)
                    else:
                        for half in range(2):
                            mt = pair * 2 + half
                            ps2 = psum.tile([128, 1024], f32, tag="g2",
                                            bufs=3,
                                            name=f"ps2_{c}_{mt}")
                            for kk in range(KK):
                                for vc in range(2):
                                    nc.tensor.matmul(
                                        ps2[:, vc * 512:(vc + 1) * 512],
                                        lhsT=hts[kk][:, mt * 128:(mt + 1) * 128],
                                        rhs=w2_all[:, kk * V + vc * 512:
                                                   kk * V + (vc + 1) * 512],
                                        start=(kk == 0), stop=(kk == KK - 1),
                                    )
                            nc.vector.tensor_add(
                                out=ot[:, half * V:(half + 1) * V],
                                in0=ps2, in1=b2_t)
                    r0 = c * 1024 + pair * 256
                    ot_ap = ot[:, :]
                    if last_pair:
                        # four small stores on alternating queues so the
                        # final transfer isn't one long serial DMA
                        qs = (nc.sync, nc.gpsimd, nc.scalar, nc.sync)
                        for j in range(4):
                            half, vc = divmod(j, 2)
                            i2 = bass.AP(tensor=ot_ap.tensor,
                                         offset=ot_ap.offset + half * V + vc * 512,
                                         ap=[ot_ap.ap[0], [1, 512]])
                            o2 = out[r0 + half * 128:r0 + (half + 1) * 128,
                                     vc * 512:(vc + 1) * 512]
                            qs[j].dma_start(out=o2, in_=i2)
                    else:
                        in3 = bass.AP(tensor=ot_ap.tensor, offset=ot_ap.offset,
                                      ap=[ot_ap.ap[0], [V, 2], [1, V]])
                        o3 = bass.AP(tensor=out.tensor, offset=r0 * V,
                                     ap=[[V, 128], [128 * V, 2], [1, V]])
                        q = (nc.sync, nc.gpsimd, nc.scalar)[(c * 4 + pair) % 3]
                        q.dma_start(out=o3, in_=in3)

    nc.finalize()
    return nc


def _get_nc():
    if "nc" not in _CACHE:
        _CACHE["nc"] = _build_bass()
    return _CACHE["nc"]


def _pack_w1_half(w1_half):
    """[HID, H] -> [128, KK*4*128] where chunk kk holds the 4 k-tiles of
    lhsT (partitions = the 2H contraction dim)."""
    arr = w1_half.reshape(KK, 128, 4, 128)  # [kk, q(out), k, p(contract)]
    return np.ascontiguousarray(
        arr.transpose(3, 0, 2, 1).reshape(128, KK * 4 * 128))


def _make_in_maps(h_enc, h_dec, W1, b1, W2, b2):
    h_enc = np.asarray(h_enc, dtype=np.float32)
    h_dec = np.asarray(h_dec, dtype=np.float32)
    W1 = np.asarray(W1, dtype=np.float32)
    b1 = np.asarray(b1, dtype=np.float32)
    W2 = np.asarray(W2, dtype=np.float32)
    b2 = np.asarray(b2, dtype=np.float32)

    w1eP = _pack_w1_half(W1[:, :H]).astype(BF16)
    w1dP = _pack_w1_half(W1[:, H:]).astype(BF16)
    w2T = np.ascontiguousarray(W2.T)
    w2P = np.concatenate([w2T[kk * 128:(kk + 1) * 128, :] for kk in range(KK)],
                         axis=1).astype(BF16)
    b1P = np.ascontiguousarray(b1.reshape(KK, 128).T)  # [128, KK] f32
    b2P = np.ascontiguousarray(b2.reshape(1, V)).astype(BF16)

    in_maps = []
    for b in range(N_CORES):
        hencT = np.ascontiguousarray(h_enc[b].T)  # [H, T]
        hencP = hencT.reshape(4, 128, T).transpose(1, 0, 2).reshape(
            128, 4 * T).astype(BF16)
        hdecT = np.ascontiguousarray(h_dec[b].T)  # [H, U]
        hdecP = hdecT.reshape(4, 128, U).transpose(1, 0, 2).reshape(
            128, 4 * U).astype(BF16)
        in_maps.append({
            "hencP": np.ascontiguousarray(hencP),
            "hdecP": np.ascontiguousarray(hdecP),
            "w1eP": w1eP,
            "w1dP": w1dP,
            "w2P": w2P,
            "b1P": b1P,
            "b2P": b2P,
        })
    return in_maps


def _run(in_maps, **kwargs):
    from concourse import bass_utils
    nc = _get_nc()
    return bass_utils.run_bass_kernel_spmd(
        nc, in_maps, core_ids=list(range(N_CORES)), **kwargs)


def kernel(h_enc, h_dec, W1, b1, W2, b2):
    in_maps = _make_in_maps(h_enc, h_dec, W1, b1, W2, b2)
    res = _run(in_maps)
    outs = [r["out"].reshape(T, U, V).astype(np.float32)
            for r in res.results]
    return np.stack(outs, axis=0)
